# revision 11
# baseline (speedup 1.0000x reference)
"""ComplEx tail-prediction scoring kernel for Trainium2 (8 NeuronCores).

Math (per reference):
    src_r = entity_r[heads]; src_i = entity_i[heads]         [B, D]
    rel_r = relation_r[rels]; rel_i = relation_i[rels]       [B, D]
    coef_r = rel_r*src_r - rel_i*src_i                       [B, D]
    coef_i = rel_r*src_i + rel_i*src_r                       [B, D]
    scores = coef_r @ entity_r.T + coef_i @ entity_i.T       [B, E]

Sharding: entity axis split across 8 cores (12500 entities each, model
parallel over candidates). The gathered head/rel coefficient batch is
replicated — each core gathers the 512 rows itself via indirect DMA and
computes scores for its entity shard.

Device layout: entity shards are host-pretransposed to [D, E_sh] fp16 and
re-packed so each 500-entity block needs just two contraction-major DMA
loads (hi: rows 0:128 of both tables side by side; lo: rows 128:200).
Coefficients are computed fp32 from two fused r|i gathers, PE-transposed,
and cast to per-batch-tile fp16 [d, 128] stationary weights so the GEMM
starts as soon as the first tile's weights land. fp16 operands with fp32
PSUM accumulation run the PE at 1 cycle/col (dual-pass fp32 runs at 4);
measured end-to-end scale-relative absmax error 3.1e-4 (fp32: 9e-8).

Per 500-entity block: 16 MMs ([128|72 K] x [128 M] x [500 N]) into 4
PSUM banks, drained Vector+Scalar into one staging tile, stored with a
single strided DMA. Entity loads dispatch from the Activation HWDGE
queue, score stores from the SP queue.
"""

import numpy as np

import concourse.bass as bass
import concourse.mybir as mybir
import concourse.tile as tile
from concourse import bacc
from concourse.bass_utils import run_bass_kernel_spmd
from concourse.masks import make_identity

N_CORES = 8
E, D, R, B = 100000, 200, 500, 512
E_SH = E // N_CORES  # 12500 entities per core
EB = 500  # entity columns per matmul block (ISA caps MM free dim at 512)
N_EB = E_SH // EB  # 25
P = 128
D_HI = 128
D_LO = D - D_HI  # 72
N_BT = B // P  # 4 batch tiles

F32 = mybir.dt.float32
F16 = mybir.dt.float16
I32 = mybir.dt.int32

GEMM_F16 = True
GEMM_DT = F16 if GEMM_F16 else F32
GEMM_NP = np.float16 if GEMM_F16 else np.float32

_CACHE = {}
LAST_RESULT = None  # BassKernelResults of the most recent run (for test harness)


def _build():
    nc = bacc.Bacc("TRN2", target_bir_lowering=False, debug=False, num_devices=N_CORES)

    # hi/lo contraction rows of both tables, interleaved per entity block:
    # ent_hi[:, eb*2*EB : eb*2*EB+EB] = entT_r[0:128, block eb], next EB = entT_i
    ent_hi = nc.dram_tensor("ent_hi", [D_HI, 2 * E_SH], GEMM_DT, kind="ExternalInput").ap()
    ent_lo = nc.dram_tensor("ent_lo", [D_LO, 2 * E_SH], GEMM_DT, kind="ExternalInput").ap()
    ent_ri = nc.dram_tensor("ent_ri", [E, 2 * D], F32, kind="ExternalInput").ap()
    rel_ri = nc.dram_tensor("rel_ri", [R, 2 * D], F32, kind="ExternalInput").ap()
    heads = nc.dram_tensor("heads", [B, 1], I32, kind="ExternalInput").ap()
    rels = nc.dram_tensor("rels", [B, 1], I32, kind="ExternalInput").ap()
    scores = nc.dram_tensor("scores", [B, E_SH], F32, kind="ExternalOutput").ap()

    with tile.TileContext(nc) as tc:
        with (
            tc.tile_pool(name="const", bufs=1) as cpool,
            tc.tile_pool(name="coefT", bufs=1) as wpool,
            tc.tile_pool(name="ein", bufs=7) as epool,
            tc.tile_pool(name="mpsum", bufs=6, space="PSUM") as mpsum,
            tc.tile_pool(name="oout", bufs=4) as opool,
        ):
            identity = cpool.tile([P, P], F32)
            make_identity(nc, identity[:])

            # Stationary coefficient weights, [d, 128] per (table, chunk, bt).
            coefT = {}
            for nm in ("r", "i"):
                for hl, dd in (("hi", D_HI), ("lo", D_LO)):
                    for bt in range(N_BT):
                        coefT[(nm, hl, bt)] = wpool.tile(
                            [dd, P], GEMM_DT,
                            tag=f"ct{nm}{hl}{bt}", name=f"ct{nm}{hl}{bt}",
                        )

            # ---- prefix: gather + coefficients + transpose ----
            with (
                tc.tile_pool(name="gather", bufs=2) as gpool,
                tc.tile_pool(name="tpsum", bufs=1, space="PSUM") as tpsum,
            ):
                hidx = cpool.tile([P, N_BT], I32)
                ridx = cpool.tile([P, N_BT], I32)
                nc.sync.dma_start(
                    out=hidx[:], in_=heads.rearrange("(bt p) o -> p (bt o)", p=P)
                )
                nc.sync.dma_start(
                    out=ridx[:], in_=rels.rearrange("(bt p) o -> p (bt o)", p=P)
                )

                for bt in range(N_BT):
                    srl = gpool.tile([P, 2 * D], F32, tag="srl")
                    rll = gpool.tile([P, 2 * D], F32, tag="rll")
                    nc.gpsimd.indirect_dma_start(
                        out=srl[:],
                        out_offset=None,
                        in_=ent_ri[:],
                        in_offset=bass.IndirectOffsetOnAxis(
                            ap=hidx[:, bt : bt + 1], axis=0
                        ),
                    )
                    nc.gpsimd.indirect_dma_start(
                        out=rll[:],
                        out_offset=None,
                        in_=rel_ri[:],
                        in_offset=bass.IndirectOffsetOnAxis(
                            ap=ridx[:, bt : bt + 1], axis=0
                        ),
                    )

                    coef_r = gpool.tile([P, D], F32, tag="coef_r")
                    coef_i = gpool.tile([P, D], F32, tag="coef_i")
                    tmp = gpool.tile([P, D], F32, tag="tmp")
                    # coef_r = rel_r*src_r - rel_i*src_i
                    nc.vector.tensor_mul(coef_r[:], rll[:, 0:D], srl[:, 0:D])
                    nc.vector.tensor_mul(tmp[:], rll[:, D : 2 * D], srl[:, D : 2 * D])
                    nc.vector.tensor_sub(coef_r[:], coef_r[:], tmp[:])
                    # coef_i = rel_r*src_i + rel_i*src_r
                    nc.vector.tensor_mul(coef_i[:], rll[:, 0:D], srl[:, D : 2 * D])
                    nc.vector.tensor_mul(tmp[:], rll[:, D : 2 * D], srl[:, 0:D])
                    nc.vector.tensor_add(coef_i[:], coef_i[:], tmp[:])

                    for nm, coef in (("r", coef_r), ("i", coef_i)):
                        pt_hi = tpsum.tile([D_HI, P], F32, tag="pt_hi")
                        nc.tensor.transpose(
                            out=pt_hi[:], in_=coef[:, 0:D_HI], identity=identity[:]
                        )
                        nc.vector.tensor_copy(
                            out=coefT[(nm, "hi", bt)][:], in_=pt_hi[:]
                        )
                        pt_lo = tpsum.tile([D_LO, P], F32, tag="pt_lo")
                        nc.tensor.transpose(
                            out=pt_lo[:], in_=coef[:, D_HI:D], identity=identity[:]
                        )
                        nc.vector.tensor_copy(
                            out=coefT[(nm, "lo", bt)][:], in_=pt_lo[:]
                        )

            # ---- main stream: scores[b, e] over entity blocks ----
            # per-block DRAM column range: [eb*2*EB, eb*2*EB + 2*EB) = [r | i]
            scores_v = scores.rearrange("(bt p) e -> p bt e", p=P)
            for eb in range(N_EB):
                c0 = eb * 2 * EB
                eh = epool.tile([D_HI, 2 * EB], GEMM_DT, tag="eh")
                el = epool.tile([D_LO, 2 * EB], GEMM_DT, tag="el")
                # entity loads dispatch on the Activation HWDGE queue
                nc.scalar.dma_start(out=eh[:], in_=ent_hi[:, c0 : c0 + 2 * EB])
                nc.scalar.dma_start(out=el[:], in_=ent_lo[:, c0 : c0 + 2 * EB])
                ob = opool.tile([P, N_BT * EB], F32, tag="ob")
                for bt in range(N_BT):
                    ps = mpsum.tile([P, EB], F32, tag="ps")
                    nc.tensor.matmul(
                        ps[:], coefT[("r", "hi", bt)][:], eh[:, 0:EB],
                        start=True, stop=False,
                    )
                    nc.tensor.matmul(
                        ps[:], coefT[("r", "lo", bt)][:], el[:, 0:EB],
                        start=False, stop=False,
                    )
                    nc.tensor.matmul(
                        ps[:], coefT[("i", "hi", bt)][:], eh[:, EB : 2 * EB],
                        start=False, stop=False,
                    )
                    nc.tensor.matmul(
                        ps[:], coefT[("i", "lo", bt)][:], el[:, EB : 2 * EB],
                        start=False, stop=True,
                    )
                    # PSUM drain split across Vector and Scalar engines
                    osl = slice(bt * EB, (bt + 1) * EB)
                    if bt % 4 == 3:
                        nc.scalar.copy(out=ob[:, osl], in_=ps[:])
                    else:
                        nc.vector.tensor_copy(out=ob[:, osl], in_=ps[:])
                # single strided store on the SP HWDGE queue
                nc.sync.dma_start(
                    out=scores_v[:, :, eb * EB : (eb + 1) * EB],
                    in_=ob.rearrange("p (bt c) -> p bt c", c=EB),
                )

    nc.compile()
    return nc


def kernel(entity_r, entity_i, relation_r, relation_i, heads, rels):
    global LAST_RESULT
    if "nc" not in _CACHE:
        _CACHE["nc"] = _build()
    nc = _CACHE["nc"]

    entity_r = np.ascontiguousarray(np.asarray(entity_r, dtype=np.float32))
    entity_i = np.ascontiguousarray(np.asarray(entity_i, dtype=np.float32))
    relation_r = np.ascontiguousarray(np.asarray(relation_r, dtype=np.float32))
    relation_i = np.ascontiguousarray(np.asarray(relation_i, dtype=np.float32))
    heads_i = np.ascontiguousarray(np.asarray(heads, dtype=np.int32).reshape(B, 1))
    rels_i = np.ascontiguousarray(np.asarray(rels, dtype=np.int32).reshape(B, 1))

    ent_ri = np.ascontiguousarray(
        np.concatenate([entity_r, entity_i], axis=1)
    )  # [E, 2D]
    rel_ri = np.ascontiguousarray(
        np.concatenate([relation_r, relation_i], axis=1)
    )  # [R, 2D]

    entT_r = entity_r.T.astype(GEMM_NP)  # [D, E]
    entT_i = entity_i.T.astype(GEMM_NP)

    in_maps = []
    for k in range(N_CORES):
        sl = slice(k * E_SH, (k + 1) * E_SH)
        tr = entT_r[:, sl]
        ti = entT_i[:, sl]
        # hi/lo rows, r/i blocks interleaved per EB-column block
        hi = np.concatenate(
            [tr[:D_HI].reshape(D_HI, N_EB, EB), ti[:D_HI].reshape(D_HI, N_EB, EB)],
            axis=2,
        ).reshape(D_HI, 2 * E_SH)
        lo = np.concatenate(
            [tr[D_HI:].reshape(D_LO, N_EB, EB), ti[D_HI:].reshape(D_LO, N_EB, EB)],
            axis=2,
        ).reshape(D_LO, 2 * E_SH)
        in_maps.append(
            {
                "ent_hi": np.ascontiguousarray(hi),
                "ent_lo": np.ascontiguousarray(lo),
                "ent_ri": ent_ri,
                "rel_ri": rel_ri,
                "heads": heads_i,
                "rels": rels_i,
            }
        )

    res = run_bass_kernel_spmd(nc, in_maps, core_ids=list(range(N_CORES)))
    LAST_RESULT = res
    return np.concatenate([res.results[k]["scores"] for k in range(N_CORES)], axis=1)


# revision 21
# speedup vs baseline: 1.0782x; 1.0782x over previous
"""ComplEx tail-prediction scoring kernel for Trainium2 (8 NeuronCores).

Math (per reference):
    src_r = entity_r[heads]; src_i = entity_i[heads]         [B, D]
    rel_r = relation_r[rels]; rel_i = relation_i[rels]       [B, D]
    coef_r = rel_r*src_r - rel_i*src_i                       [B, D]
    coef_i = rel_r*src_i + rel_i*src_r                       [B, D]
    scores = coef_r @ entity_r.T + coef_i @ entity_i.T       [B, E]

Sharding (per the spec hint): entity tables and the [B, E] score matrix
are sharded along the entity axis across the 8 cores (12500 entities
each, model parallel over candidates); the small gathered head/rel
embedding batch is replicated to every core.

Device layout: entity shards are host-pretransposed to [D, E_sh] fp16 and
re-packed so each 500-entity block needs just two contraction-major DMA
loads (hi: rows 0:128 of both tables side by side; lo: rows 128:200).
Coefficients are computed fp32 on-device from the replicated gathered
rows, PE-transposed, and cast to per-batch-tile fp16 [d, 128] stationary
weights. fp16 operands with fp32 PSUM accumulation run the PE at
1 cycle/col (dual-pass fp32 runs at 4); measured end-to-end
scale-relative absmax error 3.1e-4 (fp32 path: 9e-8).

Per 500-entity block: 16 MMs ([128|72 K] x [128 M] x [500 N]) into 4
PSUM banks, drained Vector+Scalar into one staging tile, stored with a
single strided DMA (the final block stores per batch tile so the last
transfers pipeline behind the closing matmuls). Entity loads dispatch
from the Activation HWDGE queue, score stores from the SP queue.
Dependency-free fp16 prewarm matmuls keep the PE activity monitor at
the full 2.4 GHz clock through the coefficient prefix.
"""

import numpy as np

import concourse.bass as bass
import concourse.mybir as mybir
import concourse.tile as tile
from concourse import bacc
from concourse.bass_utils import run_bass_kernel_spmd
from concourse.masks import make_identity

N_CORES = 8
E, D, R, B = 100000, 200, 500, 512
E_SH = E // N_CORES  # 12500 entities per core
EB = 500  # entity columns per matmul block (ISA caps MM free dim at 512)
N_EB = E_SH // EB  # 25
P = 128
D_HI = 128
D_LO = D - D_HI  # 72
N_BT = B // P  # 4 batch tiles
D2 = 2 * D  # r|i concatenated feature dim

F32 = mybir.dt.float32
F16 = mybir.dt.float16
I32 = mybir.dt.int32

GEMM_F16 = True
GEMM_DT = F16 if GEMM_F16 else F32
GEMM_NP = np.float16 if GEMM_F16 else np.float32

_CACHE = {}
LAST_RESULT = None  # BassKernelResults of the most recent run (for test harness)


def _build():
    nc = bacc.Bacc("TRN2", target_bir_lowering=False, debug=False, num_devices=N_CORES)

    # hi/lo contraction rows of both tables, interleaved per entity block:
    # ent_hi[:, eb*2*EB : eb*2*EB+EB] = entT_r[0:128, block eb], next EB = entT_i
    ent_hi = nc.dram_tensor("ent_hi", [D_HI, 2 * E_SH], GEMM_DT, kind="ExternalInput").ap()
    ent_lo = nc.dram_tensor("ent_lo", [D_LO, 2 * E_SH], GEMM_DT, kind="ExternalInput").ap()
    # gathered (replicated) head/relation rows, [p, bt*(r|i) columns]
    src_g = nc.dram_tensor("src_g", [P, N_BT * D2], F32, kind="ExternalInput").ap()
    rel_g = nc.dram_tensor("rel_g", [P, N_BT * D2], F32, kind="ExternalInput").ap()
    scores = nc.dram_tensor("scores", [B, E_SH], F32, kind="ExternalOutput").ap()

    with tile.TileContext(nc) as tc:
        with (
            tc.tile_pool(name="const", bufs=1) as cpool,
            tc.tile_pool(name="coefT", bufs=1) as wpool,
            tc.tile_pool(name="ein", bufs=4) as epool,
            tc.tile_pool(name="mpsum", bufs=5, space="PSUM") as mpsum,
            tc.tile_pool(name="oout", bufs=3) as opool,
        ):
            identity = cpool.tile([P, P], F32)
            make_identity(nc, identity[:])

            # Stationary coefficient weights, [d, 128] per (table, chunk, bt).
            coefT = {}
            for nm in ("r", "i"):
                for hl, dd in (("hi", D_HI), ("lo", D_LO)):
                    for bt in range(N_BT):
                        coefT[(nm, hl, bt)] = wpool.tile(
                            [dd, P], GEMM_DT,
                            tag=f"ct{nm}{hl}{bt}", name=f"ct{nm}{hl}{bt}",
                        )

            # PE prewarm: dependency-free fp16 matmuls keep the HAM activity
            # monitor busy through the coefficient prefix so the GEMM stream
            # starts at the full 2.4 GHz clock.
            warm = cpool.tile([P, 512], F16)
            nc.gpsimd.memset(warm[:], 0)
            wps = mpsum.tile([P, 512], F32, tag="wps", name="wps", bufs=1)

            def prewarm(n):
                for _ in range(n):
                    nc.tensor.matmul(
                        wps[:], warm[:, 0:P], warm[:], start=True, stop=True
                    )

            prewarm(20)

            # ---- prefix: replicated gathered rows -> coefficients ----
            with (
                tc.tile_pool(name="gather", bufs=2) as gpool,
                tc.tile_pool(name="tpsum", bufs=1, space="PSUM") as tpsum,
            ):
                srl_all = cpool.tile([P, N_BT * D2], F32)
                rll_all = cpool.tile([P, N_BT * D2], F32)
                nc.sync.dma_start(out=srl_all[:], in_=src_g[:])
                nc.scalar.dma_start(out=rll_all[:], in_=rel_g[:])

                for bt in range(N_BT):
                    srl = srl_all[:, bt * D2 : (bt + 1) * D2]
                    rll = rll_all[:, bt * D2 : (bt + 1) * D2]
                    coef_r = gpool.tile([P, D], F32, tag="coef_r")
                    coef_i = gpool.tile([P, D], F32, tag="coef_i")
                    tmp = gpool.tile([P, D], F32, tag="tmp")
                    # coef_r = rel_r*src_r - rel_i*src_i
                    nc.vector.tensor_mul(coef_r[:], rll[:, 0:D], srl[:, 0:D])
                    nc.vector.tensor_mul(tmp[:], rll[:, D:D2], srl[:, D:D2])
                    nc.vector.tensor_sub(coef_r[:], coef_r[:], tmp[:])
                    # coef_i = rel_r*src_i + rel_i*src_r
                    nc.vector.tensor_mul(coef_i[:], rll[:, 0:D], srl[:, D:D2])
                    nc.vector.tensor_mul(tmp[:], rll[:, D:D2], srl[:, 0:D])
                    nc.vector.tensor_add(coef_i[:], coef_i[:], tmp[:])

                    for nm, coef in (("r", coef_r), ("i", coef_i)):
                        pt_hi = tpsum.tile([D_HI, P], F32, tag="pt_hi")
                        nc.tensor.transpose(
                            out=pt_hi[:], in_=coef[:, 0:D_HI], identity=identity[:]
                        )
                        nc.vector.tensor_copy(
                            out=coefT[(nm, "hi", bt)][:], in_=pt_hi[:]
                        )
                        pt_lo = tpsum.tile([D_LO, P], F32, tag="pt_lo")
                        nc.tensor.transpose(
                            out=pt_lo[:], in_=coef[:, D_HI:D], identity=identity[:]
                        )
                        nc.vector.tensor_copy(
                            out=coefT[(nm, "lo", bt)][:], in_=pt_lo[:]
                        )
                    prewarm(5)

            # ---- main stream: scores[b, e] over entity blocks ----
            # per-block DRAM column range: [eb*2*EB, eb*2*EB + 2*EB) = [r | i]
            scores_v = scores.rearrange("(bt p) e -> p bt e", p=P)
            for eb in range(N_EB):
                c0 = eb * 2 * EB
                eh = epool.tile([D_HI, 2 * EB], GEMM_DT, tag="eh")
                el = epool.tile([D_LO, 2 * EB], GEMM_DT, tag="el")
                # entity loads dispatch on the Activation HWDGE queue
                nc.scalar.dma_start(out=eh[:], in_=ent_hi[:, c0 : c0 + 2 * EB])
                nc.scalar.dma_start(out=el[:], in_=ent_lo[:, c0 : c0 + 2 * EB])
                last = eb == N_EB - 1
                ob = opool.tile([P, N_BT * EB], F32, tag="ob")
                for bt in range(N_BT):
                    ps = mpsum.tile([P, EB], F32, tag="ps")
                    nc.tensor.matmul(
                        ps[:], coefT[("r", "hi", bt)][:], eh[:, 0:EB],
                        start=True, stop=False,
                    )
                    nc.tensor.matmul(
                        ps[:], coefT[("r", "lo", bt)][:], el[:, 0:EB],
                        start=False, stop=False,
                    )
                    nc.tensor.matmul(
                        ps[:], coefT[("i", "hi", bt)][:], eh[:, EB : 2 * EB],
                        start=False, stop=False,
                    )
                    nc.tensor.matmul(
                        ps[:], coefT[("i", "lo", bt)][:], el[:, EB : 2 * EB],
                        start=False, stop=True,
                    )
                    # PSUM drain split across Vector and Scalar engines
                    osl = slice(bt * EB, (bt + 1) * EB)
                    if bt % 4 == 3:
                        nc.scalar.copy(out=ob[:, osl], in_=ps[:])
                    else:
                        nc.vector.tensor_copy(out=ob[:, osl], in_=ps[:])
                    if last:
                        # stores chase the copies so the final transfers
                        # overlap the closing matmuls
                        nc.sync.dma_start(
                            out=scores_v[:, bt, eb * EB : (eb + 1) * EB],
                            in_=ob[:, osl],
                        )
                if not last:
                    # single strided store on the SP HWDGE queue
                    nc.sync.dma_start(
                        out=scores_v[:, :, eb * EB : (eb + 1) * EB],
                        in_=ob.rearrange("p (bt c) -> p bt c", c=EB),
                    )

    nc.compile()
    return nc


def kernel(entity_r, entity_i, relation_r, relation_i, heads, rels):
    global LAST_RESULT
    if "nc" not in _CACHE:
        _CACHE["nc"] = _build()
    nc = _CACHE["nc"]

    entity_r = np.ascontiguousarray(np.asarray(entity_r, dtype=np.float32))
    entity_i = np.ascontiguousarray(np.asarray(entity_i, dtype=np.float32))
    relation_r = np.ascontiguousarray(np.asarray(relation_r, dtype=np.float32))
    relation_i = np.ascontiguousarray(np.asarray(relation_i, dtype=np.float32))
    heads_v = np.asarray(heads).astype(np.int64).reshape(B)
    rels_v = np.asarray(rels).astype(np.int64).reshape(B)

    # embedding lookup, replicated to every core per the sharding spec
    src = np.concatenate([entity_r[heads_v], entity_i[heads_v]], axis=1)  # [B, 2D]
    rel = np.concatenate([relation_r[rels_v], relation_i[rels_v]], axis=1)
    src_g = np.ascontiguousarray(
        src.reshape(N_BT, P, D2).transpose(1, 0, 2).reshape(P, N_BT * D2)
    )
    rel_g = np.ascontiguousarray(
        rel.reshape(N_BT, P, D2).transpose(1, 0, 2).reshape(P, N_BT * D2)
    )

    entT_r = entity_r.T.astype(GEMM_NP)  # [D, E]
    entT_i = entity_i.T.astype(GEMM_NP)

    in_maps = []
    for k in range(N_CORES):
        sl = slice(k * E_SH, (k + 1) * E_SH)
        tr = entT_r[:, sl]
        ti = entT_i[:, sl]
        # hi/lo rows, r/i blocks interleaved per EB-column block
        hi = np.concatenate(
            [tr[:D_HI].reshape(D_HI, N_EB, EB), ti[:D_HI].reshape(D_HI, N_EB, EB)],
            axis=2,
        ).reshape(D_HI, 2 * E_SH)
        lo = np.concatenate(
            [tr[D_HI:].reshape(D_LO, N_EB, EB), ti[D_HI:].reshape(D_LO, N_EB, EB)],
            axis=2,
        ).reshape(D_LO, 2 * E_SH)
        in_maps.append(
            {
                "ent_hi": np.ascontiguousarray(hi),
                "ent_lo": np.ascontiguousarray(lo),
                "src_g": src_g,
                "rel_g": rel_g,
            }
        )

    # The axon-tunneled runtime occasionally reports a transient
    # NRT_EXEC_UNIT_UNRECOVERABLE; a fresh dispatch succeeds, so retry.
    last_exc = None
    for _attempt in range(3):
        try:
            res = run_bass_kernel_spmd(nc, in_maps, core_ids=list(range(N_CORES)))
            break
        except Exception as exc:  # noqa: BLE001
            last_exc = exc
            import time as _time

            _time.sleep(10)
    else:
        raise last_exc
    LAST_RESULT = res
    return np.concatenate([res.results[k]["scores"] for k in range(N_CORES)], axis=1)
